# revision 1
# baseline (speedup 1.0000x reference)
"""Category-specific 2-layer MLP (MoE-style routing), expert-parallel on 8 NeuronCores.

Math (per sample b with category c = cat_ids[b]):
    h   = relu(x_flat[b] @ W1[c] + b1[c])      x_flat: [32, 4096], W1: [8, 4096, 1024]
    out = h @ W2[c] + b2[c]                    W2: [8, 1024, 512]

Sharding: expert-parallel. Core k holds ONLY category k's weights (16 MB W1 + 2 MB W2)
and computes the full dense MLP for all 32 samples; the host then gathers row b from
core cat_ids[b]. Per-core HBM traffic is ~18.6 MB (the minimum possible when all 8
categories are in use), vs 144 MB for weight replication.

Kernel layout per core (fp32 matmuls stream the MOVING operand at 4 cycles/row, so
keep the moving dim small: stream x^T / h^T at N=32, keep the big weights stationary):
  layer 1: hT[u] [128, 32] (u = 0..7 mid-tiles, one PSUM bank each) accumulated over
           32 K-tiles: lhsT (stationary) = W1[128t:128t+128, 128u:128u+128],
           rhs (moving) = x^T tile [128, 32]. Produces h already transposed for
           layer 2 — no on-chip transpose stage at all.
  bias+relu: ONE DVE scalar_tensor_tensor per mid-tile:
           ht_sb = max(hT_psum + b1T[:, u], 0)   (b1 transposed is per-PARTITION).
  layer 2: transposed too: oT[v] [128, 32] (v = 0..3) over 8 K-tiles:
           lhsT = W2[128u:128u+128, 128v:128v+128], rhs = hT[u] [128, 32];
           evict fuses the b2 add. Output leaves the chip as out^T [512, 32];
           the host gather undoes the transpose for free.
  W1 streams as 7 uneven DMAs ([8,8,8,4,2,1,1] K-tiles) — big slabs amortize
  per-DMA latency, the tiny last slab shortens the post-stream PE tail.

Toolchain constraint: this walrus build allows at most ONE sync-wait command per
instruction. The program is structured so every instruction acquires at most one
new semaphore:
  - every W1/W2 slab lives in its own SBUF tile (no slot reuse -> DMAs carry no waits);
  - the xt DMA is placed 8 positions before the first W1 slab DMA, so both land on the
    same HWDGE queue and one cumulative wait covers both;
  - a leading DVE "touch" of the bias tile acquires its queue semaphore before the
    fused bias ops (which then wait only on PE);
  - the kernel-tail drain is split into single-wait drains (_patch_tail_drain).
Verified by _assert_wait_budget at build time.
"""

import numpy as np

import concourse.bass as bass
import concourse.mybir as mybir
import concourse.tile_sem_assignment as _tsa
from concourse import tile
from concourse.bass_utils import run_bass_kernel_spmd

NUM_CAT = 8
B = 32
IN_DIM = 4096   # 16 * 256
MID = 1024
OUT = 512       # 16 * 32
P = 128
KT1 = IN_DIM // P    # 32 k-tiles for layer 1
KT2 = MID // P       # 8 mid-tiles (layer-1 out / layer-2 contraction)
NT = OUT // P        # 4 out-tiles
SLAB_SIZES = (8, 8, 8, 4, 2, 1, 1)  # k-tiles per W1 DMA; sum == KT1
F32 = mybir.dt.float32

HWDGE_QUEUES = 4


class _PatchHwdgeQueues:
    """Pin Tile's HWDGE round-robin to n queues during scheduling."""

    def __init__(self, n: int):
        self.n = n

    def __enter__(self):
        self._saved = _tsa.NUM_HWDGE_SEMS
        _tsa.NUM_HWDGE_SEMS = self.n
        return self

    def __exit__(self, *exc):
        _tsa.NUM_HWDGE_SEMS = self._saved
        return False


def _patch_tail_drain():
    """Split Tile's kernel-tail drain (one wait per live proc) into a chain of
    single-wait drains: this walrus build caps sync-wait commands per instruction
    and rejects the stock multi-wait drain."""
    if getattr(tile.TileContext, "_tail_drain_patched", False):
        return
    from concourse.vector_clock import ScopedClock, VectorClock

    def _drain_and_barrier(self, tick_clock, wait_clock):
        gc = tick_clock.global_clock
        n = len(gc)
        for p in range(n):
            if gc[p] <= 0:
                continue
            sub = [0] * n
            sub[p] = gc[p]
            d = self.nc.sync.drain()
            wait_clock.add_sem_waits(d.ins, ScopedClock({None: VectorClock(sub)}))
        self.nc.all_engine_barrier()
        assert self.sems is not None
        popped = self.nc._tile_sem_poison_stack.pop()
        assert popped is self._sem_poison
        self.nc.clear_and_free_semaphores(list(self.sems.allocated().values()))
        self.nc.all_engine_barrier()

    tile.TileContext._drain_and_barrier = _drain_and_barrier
    tile.TileContext._tail_drain_patched = True


_patch_tail_drain()


def _build_nc() -> bass.Bass:
    nc = bass.Bass()

    # xt[p, t, b] = x_flat[b, t*128 + p]: K-major layout so each DMA partition line
    # is one contiguous 4 KB segment.
    xt = nc.dram_tensor("xt", [P, KT1, B], F32, kind="ExternalInput")
    w1 = nc.dram_tensor("w1", [IN_DIM, MID], F32, kind="ExternalInput")
    w2 = nc.dram_tensor("w2", [MID, OUT], F32, kind="ExternalInput")
    # biast[p, 0:8] = b1[128u + p]; [p, 8:12] = b2[128v + p]; [p, 12] = 0.
    biast = nc.dram_tensor("biast", [P, KT2 + NT + 1], F32, kind="ExternalInput")
    out = nc.dram_tensor("out", [OUT, B], F32, kind="ExternalOutput")  # transposed

    with _PatchHwdgeQueues(HWDGE_QUEUES), tile.TileContext(nc) as tc:
        with (
            tc.tile_pool(name="const", bufs=1) as const,
            tc.tile_pool(name="w1p", bufs=1) as w1p,
            tc.tile_pool(name="w2p", bufs=1) as w2p,
            tc.tile_pool(name="work", bufs=1) as work,
            tc.tile_pool(name="psum", bufs=1, space="PSUM") as psum,
        ):
            # DMA issue order fixes HWDGE queue assignment (round-robin mod 4):
            # 0:xt 1:biast 2:w2a 3:w2b 4+:w1 slabs. xt (pos 0) and w1 slab 0
            # (pos 4) share a queue -> one cumulative wait covers both for the
            # first matmul. Each queue later carries a W1 slab, so every small
            # input is covered by the slab waits PE already performs.
            xt_sb = const.tile([P, KT1, B], F32)
            nc.sync.dma_start(xt_sb[:], xt[:])
            biast_sb = const.tile([P, KT2 + NT + 1], F32)
            nc.sync.dma_start(biast_sb[:], biast[:])

            # W2 in two 1 MB DMAs; w2_sbs[h][:, f, :] is K-tile 4h+f.
            w2_sbs = []
            for h in range(2):
                w2_sb = w2p.tile([P, KT2 // 2, OUT], F32, tag=f"w2_{h}", name=f"w2sb{h}")
                nc.sync.dma_start(
                    w2_sb[:],
                    w2[P * (KT2 // 2) * h : P * (KT2 // 2) * (h + 1), :].rearrange(
                        "(f p) n -> p f n", p=P
                    ),
                )
                w2_sbs.append(w2_sb)

            # W1 as 8 uneven DMAs; w1_sbs[s][:, f, :] is K-tile (slab_start[s] + f).
            w1_sbs = []
            row = 0
            slab_of_ktile = []
            for s, sz in enumerate(SLAB_SIZES):
                w1_sb = w1p.tile([P, sz, MID], F32, tag=f"w1_{s}", name=f"w1sb{s}")
                nc.sync.dma_start(
                    w1_sb[:],
                    w1[row : row + P * sz, :].rearrange("(f p) n -> p f n", p=P),
                )
                w1_sbs.append(w1_sb)
                slab_of_ktile += [(s, f) for f in range(sz)]
                row += P * sz

            # ---- layer 1: hT[u][128, 32] = (x @ W1)^T mid-tiles, 8 PSUM banks ----
            ht_ps = [
                psum.tile([P, B], F32, tag=f"hT_{u}", name=f"htps{u}")
                for u in range(KT2)
            ]
            for t in range(KT1):
                s, f = slab_of_ktile[t]
                for u in range(KT2):
                    nc.tensor.matmul(
                        ht_ps[u][:],
                        w1_sbs[s][:, f, P * u : P * (u + 1)],
                        xt_sb[:, t, :],
                        start=(t == 0),
                        stop=(t == KT1 - 1),
                    )

            # DVE touch: acquire the biast queue semaphore ahead of the fused
            # bias ops so they only ever wait on PE.
            touch_sb = work.tile([P, 1], F32)
            nc.vector.tensor_copy(touch_sb[:], biast_sb[:, 12:13])

            zero_bc = biast_sb[:, 12:13].to_broadcast((P, B))

            # ---- fused bias+relu evict: ht_sb[:,u,:] = max(hT[u] + b1T[:,u], 0) ----
            ht_sb = work.tile([P, KT2, B], F32)
            for u in range(KT2):
                nc.vector.scalar_tensor_tensor(
                    ht_sb[:, u, :],
                    ht_ps[u][:],
                    biast_sb[:, u : u + 1],
                    zero_bc,
                    mybir.AluOpType.add,
                    mybir.AluOpType.max,
                )

            # ---- layer 2 (transposed): oT[v][128, 32] over 8 K-tiles ----
            # oT psum tiles reuse hT_0..3 slots (released once their evict ran).
            ot_ps = [
                psum.tile([P, B], F32, tag=f"hT_{v}", name=f"otps{v}")
                for v in range(NT)
            ]
            for u in range(KT2):
                for v in range(NT):
                    nc.tensor.matmul(
                        ot_ps[v][:],
                        w2_sbs[u // 4][:, u % 4, P * v : P * (v + 1)],
                        ht_sb[:, u, :],
                        start=(u == 0),
                        stop=(u == KT2 - 1),
                    )

            # ---- fused bias evict: ot_sb[:,v,:] = oT[v] + b2T[:,v] ----
            ot_sb = work.tile([P, NT, B], F32)
            for v in range(NT):
                nc.vector.scalar_tensor_tensor(
                    ot_sb[:, v, :],
                    ot_ps[v][:],
                    biast_sb[:, KT2 + v : KT2 + v + 1],
                    zero_bc,
                    mybir.AluOpType.add,
                    mybir.AluOpType.add,
                )
            # SWDGE path: a fresh DMA proc, so the store carries only the DVE wait
            # (an HWDGE queue would add a self-queue FIFO wait -> 2 waits).
            nc.gpsimd.dma_start(out.rearrange("(v p) b -> p v b", p=P), ot_sb[:])

    _assert_wait_budget(nc)
    return nc


def _assert_wait_budget(nc: bass.Bass, max_waits: int = 1):
    """This walrus build rejects instructions with >1 sync wait; fail fast."""
    bad = []
    for blk in nc.m.functions[0].blocks:
        for inst in blk.instructions:
            if type(inst).__name__ not in (
                "InstMatmult",
                "InstDMACopy",
                "InstDrain",
                "InstTensorCopy",
                "InstTensorScalarPtr",
            ):
                continue
            si = inst.sync_info
            nw = len(si.on_wait) if si is not None else 0
            if nw > max_waits:
                bad.append(
                    (
                        inst.name,
                        type(inst).__name__,
                        [(w.ant_name, w.wait_value) for w in si.on_wait],
                    )
                )
    if bad:
        raise RuntimeError(f"instructions with >{max_waits} sync waits: {bad}")


_NC_CACHE: bass.Bass | None = None


def _get_nc() -> bass.Bass:
    global _NC_CACHE
    if _NC_CACHE is None:
        _NC_CACHE = _build_nc()
    return _NC_CACHE


def _make_in_maps(x, W1, b1, W2, b2):
    x_flat = np.ascontiguousarray(np.asarray(x, dtype=np.float32)).reshape(B, IN_DIM)
    # xt[p, t, b] = x_flat[b, t*128 + p]
    xt = np.ascontiguousarray(x_flat.reshape(B, KT1, P).transpose(2, 1, 0))
    W1 = np.ascontiguousarray(np.asarray(W1, dtype=np.float32))
    W2 = np.ascontiguousarray(np.asarray(W2, dtype=np.float32))
    b1 = np.asarray(b1, dtype=np.float32)
    b2 = np.asarray(b2, dtype=np.float32)
    biast = np.zeros((NUM_CAT, P, KT2 + NT + 1), dtype=np.float32)
    biast[:, :, :KT2] = b1.reshape(NUM_CAT, KT2, P).transpose(0, 2, 1)
    biast[:, :, KT2 : KT2 + NT] = b2.reshape(NUM_CAT, NT, P).transpose(0, 2, 1)
    return [
        {
            "xt": xt,
            "w1": W1[k],
            "w2": W2[k],
            "biast": biast[k],
        }
        for k in range(NUM_CAT)
    ]


def kernel(x, W1, b1, W2, b2, cat_ids) -> np.ndarray:
    nc = _get_nc()
    in_maps = _make_in_maps(x, W1, b1, W2, b2)
    res = run_bass_kernel_spmd(nc, in_maps, list(range(NUM_CAT))).results
    per_cat = np.stack([np.asarray(res[k]["out"]) for k in range(NUM_CAT)])  # [8, OUT, B]
    cat = np.asarray(cat_ids).astype(np.int64).reshape(B)
    sel = per_cat[cat, :, np.arange(B)]  # [B, OUT] (gather undoes the transpose)
    return np.ascontiguousarray(sel.reshape(B, 16, 32).astype(np.float32))



# revision 9
# speedup vs baseline: 5.5081x; 5.5081x over previous
"""Category-specific 2-layer MLP (MoE-style routing), expert-parallel on 8 NeuronCores.

Math (per sample b with category c = cat_ids[b]):
    h   = relu(x_flat[b] @ W1[c] + b1[c])      x_flat: [32, 4096], W1: [8, 4096, 1024]
    out = h @ W2[c] + b2[c]                    W2: [8, 1024, 512]

Sharding: expert-parallel. Core k holds ONLY category k's weights and computes the
full dense MLP for all 32 samples; the host gathers row b from core cat_ids[b].

Per-core traffic is minimized with precision folding (rel-err budget is 2e-2):
  W1 -> fp8 e3m4 scaled by 128   (measured end-to-end rel err ~1.4e-2)
  x  -> bf16 scaled by 1/128     (power-of-2 scales cancel exactly in the matmul)
  W2 -> bf16
This cuts the DMA stream from ~19.4 MB fp32 to ~5.6 MB.

The stream is split over the THREE procs that can issue DMAs (SP + Activation via
HWDGE, Pool via SWDGE): in this toolchain's cost model each DMA's transfer time is
charged to the issuing engine, so three engines give three parallel streams.
NUM_HWDGE_SEMS is pinned to 2 with strictly alternating SP/Act issue order so each
engine owns one HWDGE proc (per-proc completion stays FIFO, keeping cumulative
semaphore waits sound).

Layer 1 computes hT[u] = (x @ W1)^T mid-tiles into a single PSUM tile [128, 8, 32]
(8 interleaved accumulation groups); biases (when nonzero) are folded in as an
extra K=1 matmul row (lhsT = bias segment, rhs = ones) so the PSUM evict is a
single tensor_scalar max(psum, 0) -> bf16. Layer 2 likewise, evicted by a single
tensor_copy, then stored transposed via SWDGE; the host gather undoes the
transpose for free.

The PE clock ramps to full speed only after 3us of continuous busy; any idle gap
resets it. Dummy [1, N] matmuls on a memset tile warm the PE up during the DMA
stream and bridge predicted arrival gaps so the real matmuls run at full clock.

Toolchain constraint: at most ONE sync-wait per instruction. Every W1/W2 slab has
its own SBUF tile (no slot-reuse waits); xt is issued before the W1 slabs on the
Act proc and one dummy matmul touches it (so later matmuls carry only their slab
wait); W2 halves are issued before the last W1 slabs on their procs so layer 2's
waits are covered transitively. Verified by _assert_wait_budget at build time.
"""

import numpy as np

import concourse.bass as bass
import concourse.mybir as mybir
import concourse.tile_sem_assignment as _tsa
from concourse import tile
from concourse.bass_utils import run_bass_kernel_spmd

NUM_CAT = 8
B = 32
IN_DIM = 4096   # 16 * 256
MID = 1024
OUT = 512       # 16 * 32
P = 128
KT1 = IN_DIM // P    # 32 k-tiles for layer 1
KT2 = MID // P       # 8 mid-tiles (layer-1 out / layer-2 contraction)
NT = OUT // P        # 4 out-tiles
S1 = 128.0           # power-of-2 scale: W1 *= S1 (fp8), x /= S1 (bf16)

F32 = mybir.dt.float32
BF16 = mybir.dt.bfloat16
FP8 = mybir.dt.float8e3  # e3m4

# --- per-engine slab plan -----------------------------------------------------
# Each entry is k-tiles per W1 DMA on that engine. SP additionally carries the
# first W2 half (inserted before its last two slabs), Act carries xt (first) and
# the second W2 half (before its last two slabs), Pool carries only W1 (and the
# output store + optional aux at the end). SP/Act must issue the SAME number of
# HWDGE DMAs, strictly alternating, so each owns one of the 2 HWDGE procs.
SP_SLABS = (2, 2, 2, 2, 2)    # 10 kt + W2a   -> 6 HWDGE DMAs
ACT_SLABS = (2, 2, 2, 2)      # 8 kt + xt+W2b -> 6 HWDGE DMAs
POOL_SLABS = (2, 2, 2, 2, 2, 2, 2)  # 14 kt via SWDGE
assert sum(SP_SLABS) + sum(ACT_SLABS) + sum(POOL_SLABS) == KT1
assert len(SP_SLABS) + 1 == len(ACT_SLABS) + 2

# --- cost-model constants used only for static PE schedule planning -----------
_NSPB = 0.3855421  # DMA ns per byte-per-partition (400e9/128 B/s * 0.83 util)
_DMA_MIN = 500.0
_HW_INIT = 1716.7  # HWDGE post-exec latency before data is consumable
_SW_INIT = 1883.3  # SWDGE (Pool) same
_SEM = 100.0
_T0 = 200.0        # engine streams start after the tile preamble barrier
_PE_MID = 1e9 / 1.2e9
_PE_FULL = 1e9 / 2.4e9
_RAMP_NS = 3000.0


def _dma_exec(bytes_pp: float) -> float:
    return max(bytes_pp * _NSPB, _DMA_MIN)


class _PatchHwdgeQueues:
    """Pin Tile's HWDGE round-robin to n procs during scheduling."""

    def __init__(self, n: int):
        self.n = n

    def __enter__(self):
        self._saved = _tsa.NUM_HWDGE_SEMS
        _tsa.NUM_HWDGE_SEMS = self.n
        return self

    def __exit__(self, *exc):
        _tsa.NUM_HWDGE_SEMS = self._saved
        return False


def _patch_tail_drain():
    """Split Tile's kernel-tail drain (one wait per live proc) into a chain of
    single-wait drains: this walrus build caps sync-wait commands per instruction
    and rejects the stock multi-wait drain."""
    if getattr(tile.TileContext, "_tail_drain_patched", False):
        return
    from concourse.vector_clock import ScopedClock, VectorClock

    def _drain_and_barrier(self, tick_clock, wait_clock):
        gc = tick_clock.global_clock
        n = len(gc)
        for p in range(n):
            if gc[p] <= 0:
                continue
            sub = [0] * n
            sub[p] = gc[p]
            d = self.nc.sync.drain()
            wait_clock.add_sem_waits(d.ins, ScopedClock({None: VectorClock(sub)}))
        self.nc.all_engine_barrier()
        assert self.sems is not None
        popped = self.nc._tile_sem_poison_stack.pop()
        assert popped is self._sem_poison
        self.nc.clear_and_free_semaphores(list(self.sems.allocated().values()))
        self.nc.all_engine_barrier()

    tile.TileContext._drain_and_barrier = _drain_and_barrier
    tile.TileContext._tail_drain_patched = True


_patch_tail_drain()


def _build_nc(with_bias: bool) -> bass.Bass:
    nc = bass.Bass()

    # xt[p, t, b] = x_flat[b, t*128 + p] / 128, bf16.
    xt = nc.dram_tensor("xt", [P, KT1, B], BF16, kind="ExternalInput")
    w1 = nc.dram_tensor("w1", [IN_DIM, MID], FP8, kind="ExternalInput")
    w2 = nc.dram_tensor("w2", [MID, OUT], BF16, kind="ExternalInput")
    if with_bias:
        # aux[0, :MID] = b1; aux[0, MID:MID+OUT] = b2; aux[0, MID+OUT:] = 1.0
        aux = nc.dram_tensor("aux", [1, MID + OUT + B], BF16, kind="ExternalInput")
    out = nc.dram_tensor("out", [OUT, B], F32, kind="ExternalOutput")  # transposed

    with _PatchHwdgeQueues(2), tile.TileContext(nc) as tc:
        with (
            tc.tile_pool(name="const", bufs=1) as const,
            tc.tile_pool(name="w1p", bufs=1) as w1p,
            tc.tile_pool(name="w2p", bufs=1) as w2p,
            tc.tile_pool(name="work", bufs=1) as work,
            tc.tile_pool(name="psum", bufs=1, space="PSUM") as psum,
        ):
            # Warmup data for PE dummy matmuls: DVE memsets it right away, so
            # dummies can start ~0.5us in and hold the PE clock ramp.
            warm_sb = work.tile([1, 257], BF16)
            nc.vector.memset(warm_sb[:], 1.0)

            # ---- DMA issue. HWDGE round-robin = python order; alternate SP/Act
            # strictly so SP DMAs own proc0 and Act DMAs own proc1. Pool DMAs
            # round-robin their own SWDGE procs independently.
            sp_q = []   # (tile, n_ktiles, row0)  in SP issue order
            act_q = []
            pool_q = []

            row = 0
            slabs = []  # (engine_name, tile, sz, row0) in global k-tile order

            def w1_slab(sz, name):
                nonlocal row
                t = w1p.tile([P, sz, MID], FP8, tag=name, name=name)
                r0 = row
                row += P * sz
                return (t, sz, r0)

            # Assign k-tile ranges engine by engine (order within engine =
            # issue order). Global k order: SP slabs, Act slabs, Pool slabs.
            sp_slabs = [w1_slab(sz, f"w1s{i}") for i, sz in enumerate(SP_SLABS)]
            act_slabs = [w1_slab(sz, f"w1a{i}") for i, sz in enumerate(ACT_SLABS)]
            pool_slabs = [w1_slab(sz, f"w1p{i}") for i, sz in enumerate(POOL_SLABS)]
            assert row == IN_DIM * 1  # all rows covered

            xt_sb = const.tile([P, KT1, B], BF16)
            w2a_sb = w2p.tile([P, KT2 // 2, OUT], BF16, tag="w2a", name="w2a")
            w2b_sb = w2p.tile([P, KT2 // 2, OUT], BF16, tag="w2b", name="w2b")
            if with_bias:
                aux_sb = const.tile([1, MID + OUT + B], BF16)

            # Interleaved HWDGE issue: (SP, Act) pairs.
            # SP:  s0 s1 s2 W2a s3 s4
            # Act: xt a0 a1 W2b a2 a3
            def dma_w1(eng, slab):
                t, sz, r0 = slab
                eng.dma_start(
                    t[:], w1[r0 : r0 + P * sz, :].rearrange("(f p) n -> p f n", p=P)
                )

            sp_seq = (
                [("w1", sp_slabs[0]), ("w1", sp_slabs[1]), ("w1", sp_slabs[2]),
                 ("w2a", None), ("w1", sp_slabs[3]), ("w1", sp_slabs[4])]
            )
            act_seq = (
                [("xt", None), ("w1", act_slabs[0]), ("w1", act_slabs[1]),
                 ("w2b", None), ("w1", act_slabs[2]), ("w1", act_slabs[3])]
            )
            assert len(sp_seq) == len(act_seq)

            for (sp_kind, sp_arg), (act_kind, act_arg) in zip(sp_seq, act_seq):
                # SP issue
                if sp_kind == "w1":
                    dma_w1(nc.sync, sp_arg)
                else:
                    nc.sync.dma_start(
                        w2a_sb[:],
                        w2[0 : P * (KT2 // 2), :].rearrange("(f p) n -> p f n", p=P),
                    )
                # Act issue
                if act_kind == "xt":
                    nc.scalar.dma_start(xt_sb[:], xt[:])
                elif act_kind == "w1":
                    dma_w1(nc.scalar, act_arg)
                else:
                    nc.scalar.dma_start(
                        w2b_sb[:],
                        w2[P * (KT2 // 2) :, :].rearrange("(f p) n -> p f n", p=P),
                    )

            # Pool (SWDGE) stream: all its W1 slabs, then aux (bias variant).
            for slab in pool_slabs:
                dma_w1(nc.gpsimd, slab)
            if with_bias:
                nc.gpsimd.dma_start(aux_sb[:], aux[:])

            # ---- static arrival-time plan (cost-model replica) --------------
            sp_t = _T0
            act_t = _T0
            pool_t = _T0

            vis = {}  # slab id -> (visible_time, tile, sz, kt0)
            kt0 = 0

            def kt_of(slab):
                t, sz, r0 = slab
                return r0 // P

            for kind, arg in sp_seq:
                if kind == "w1":
                    t, sz, r0 = arg
                    sp_t += _dma_exec(sz * MID * 1)
                    vis[id(arg)] = (sp_t + _HW_INIT + _SEM, t, sz, r0 // P)
                else:
                    sp_t += _dma_exec((KT2 // 2) * OUT * 2)
            xt_vis = None
            for kind, arg in act_seq:
                if kind == "w1":
                    t, sz, r0 = arg
                    act_t += _dma_exec(sz * MID * 1)
                    vis[id(arg)] = (act_t + _HW_INIT + _SEM, t, sz, r0 // P)
                elif kind == "xt":
                    act_t += _dma_exec(KT1 * B * 2)
                    xt_vis = act_t + _HW_INIT + _SEM
                else:
                    act_t += _dma_exec((KT2 // 2) * OUT * 2)
            for slab in pool_slabs:
                t, sz, r0 = slab
                pool_t += _dma_exec(sz * MID * 1)
                vis[id(slab)] = (pool_t + _SW_INIT + _SEM, t, sz, r0 // P)

            arrival = sorted(
                (vis[id(s)] for s in sp_slabs + act_slabs + pool_slabs),
                key=lambda e: e[0],
            )

            # ---- PE program ---------------------------------------------------
            # PSUM: hT (8 groups in one tile), oT, dummy target.
            ht_ps_lo = psum.tile([P, KT2 // 2, B], F32, tag="hTlo", name="htpslo")
            ht_ps_hi = psum.tile([P, KT2 // 2, B], F32, tag="hThi", name="htpshi")
            ot_ps = psum.tile([P, NT, B], F32, tag="oT", name="otps")
            warm_ps = psum.tile([1, 256], F32, tag="warm", name="warmps")

            pe_t = 520.0          # planned first-dummy start (memset + sem)
            pe_busy0 = pe_t
            SLACK = 90.0

            def pe_cycle():
                return _PE_FULL if (pe_t - pe_busy0) > _RAMP_NS else _PE_MID

            def dummy(n_free):
                nonlocal pe_t
                nc.tensor.matmul(
                    warm_ps[0:1, 0:n_free],
                    warm_sb[0:1, 256:257],
                    warm_sb[0:1, 0:n_free],
                    start=True,
                    stop=True,
                    skip_group_check=True,
                )
                pe_t += n_free * pe_cycle()

            def fill_until(target):
                # keep PE busy (no idle gap -> no clock-ramp reset) until target
                while pe_t < target:
                    gap = target - pe_t
                    n = int(np.clip(gap / pe_cycle() + 24, 32, 256))
                    dummy(n)

            started = set()  # u groups with start consumed

            def l1_slab(tile_, sz, kt_base, is_last):
                nonlocal pe_t
                for f in range(sz):
                    for u in range(KT2):
                        # zero-bias: the very last k-row of the last slab stops
                        # each group; bias variant stops via the bias row.
                        stops = (
                            (not with_bias) and is_last and f == sz - 1
                        )
                        ht_ps = ht_ps_lo if u < KT2 // 2 else ht_ps_hi
                        nc.tensor.matmul(
                            ht_ps[:, u % (KT2 // 2), :],
                            tile_[:, f, P * u : P * (u + 1)],
                            xt_sb[:, kt_base + f, :],
                            start=(u not in started),
                            stop=stops,
                            skip_group_check=True,
                        )
                        started.add(u)
                        pe_t += B * pe_cycle()

            # warm up until xt is visible (xt is Act's first DMA, so it is
            # visible no later than any Act slab; SP/Pool firsts are ~equal)
            fill_until(xt_vis + SLACK)
            # touch xt so later matmuls only ever wait on their own W1 slab
            nc.tensor.matmul(
                warm_ps[0:1, 0:B],
                warm_sb[0:1, 256:257],
                xt_sb[0:1, 0, :],
                start=True,
                stop=True,
                skip_group_check=True,
            )
            pe_t += B * pe_cycle()

            for i, (t_vis, tile_, sz, kt_base) in enumerate(arrival):
                fill_until(t_vis + SLACK)
                l1_slab(tile_, sz, kt_base, is_last=(i == len(arrival) - 1))

            # close the 8 accumulation groups: bias rows (or tiny stop-only
            # matmuls when biases are zero -- reuse the last k-tile row with
            # zero contribution is impossible, so use explicit stop via bias
            # path only in bias variant; here emit stop on a zero-cost pattern)
            if with_bias:
                for u in range(KT2):
                    ht_ps = ht_ps_lo if u < KT2 // 2 else ht_ps_hi
                    nc.tensor.matmul(
                        ht_ps[:, u % (KT2 // 2), :],
                        aux_sb[0:1, P * u : P * (u + 1)],
                        aux_sb[0:1, MID + OUT : MID + OUT + B],
                        start=False,
                        stop=True,
                        skip_group_check=True,
                    )
                    pe_t += B * pe_cycle()

            # ---- evict hT: single op per half (DVE + Pool run in parallel) --
            ht_lo = work.tile([P, KT2 // 2, B], BF16, tag="ht_lo", name="ht_lo")
            ht_hi = work.tile([P, KT2 // 2, B], BF16, tag="ht_hi", name="ht_hi")
            nc.vector.tensor_scalar_max(ht_lo[:], ht_ps_lo[:], 0.0)
            nc.gpsimd.tensor_scalar_max(ht_hi[:], ht_ps_hi[:], 0.0)

            # bridge the evict wait so layer 2 still sees a warm PE clock
            fill_until(pe_t + 500.0)

            # ---- layer 2: oT[v] over 8 K-tiles (+ bias row) ------------------
            for v in range(NT):
                for u in range(KT2):
                    w2_sb = w2a_sb if u < KT2 // 2 else w2b_sb
                    ht_half = ht_lo if u < KT2 // 2 else ht_hi
                    nc.tensor.matmul(
                        ot_ps[:, v, :],
                        w2_sb[:, u % (KT2 // 2), P * v : P * (v + 1)],
                        ht_half[:, u % (KT2 // 2), :],
                        start=(u == 0),
                        stop=(not with_bias) and (u == KT2 - 1),
                        skip_group_check=True,
                    )
                    pe_t += B * pe_cycle()
                if with_bias:
                    nc.tensor.matmul(
                        ot_ps[:, v, :],
                        aux_sb[0:1, MID + P * v : MID + P * (v + 1)],
                        aux_sb[0:1, MID + OUT : MID + OUT + B],
                        start=False,
                        stop=True,
                        skip_group_check=True,
                    )
                    pe_t += B * pe_cycle()

            # ---- evict oT (fp32) and store transposed via SWDGE --------------
            ot_sb = work.tile([P, NT, B], F32)
            nc.vector.tensor_copy(ot_sb[:], ot_ps[:])
            nc.gpsimd.dma_start(out.rearrange("(v p) b -> p v b", p=P), ot_sb[:])

    _assert_wait_budget(nc)
    return nc


def _assert_wait_budget(nc: bass.Bass, max_waits: int = 1):
    """This walrus build rejects instructions with >1 sync wait; fail fast."""
    bad = []
    for blk in nc.m.functions[0].blocks:
        for inst in blk.instructions:
            if type(inst).__name__ not in (
                "InstMatmult",
                "InstDMACopy",
                "InstDrain",
                "InstTensorCopy",
                "InstTensorScalarPtr",
                "InstMemset",
            ):
                continue
            si = inst.sync_info
            nw = len(si.on_wait) if si is not None else 0
            if nw > max_waits:
                bad.append(
                    (
                        inst.name,
                        type(inst).__name__,
                        [(w.ant_name, w.wait_value) for w in si.on_wait],
                    )
                )
    if bad:
        raise RuntimeError(f"instructions with >{max_waits} sync waits: {bad}")


_NC_CACHE: dict[bool, bass.Bass] = {}


def _get_nc(with_bias: bool) -> bass.Bass:
    if with_bias not in _NC_CACHE:
        _NC_CACHE[with_bias] = _build_nc(with_bias)
    return _NC_CACHE[with_bias]


def _make_in_maps(x, W1, b1, W2, b2, with_bias: bool):
    import ml_dtypes

    x_flat = np.ascontiguousarray(np.asarray(x, dtype=np.float32)).reshape(B, IN_DIM)
    # xt[p, t, b] = x_flat[b, t*128 + p] / S1  (exact: power-of-2 scale)
    xt = np.ascontiguousarray(
        (x_flat / S1).reshape(B, KT1, P).transpose(2, 1, 0)
    ).astype(ml_dtypes.bfloat16)
    W1q = np.asarray(
        np.asarray(W1, dtype=np.float32) * S1, dtype=ml_dtypes.float8_e3m4
    )
    W2q = np.asarray(np.asarray(W2, dtype=np.float32), dtype=ml_dtypes.bfloat16)
    maps = []
    for k in range(NUM_CAT):
        m = {
            "xt": xt,
            "w1": np.ascontiguousarray(W1q[k]),
            "w2": np.ascontiguousarray(W2q[k]),
        }
        if with_bias:
            auxk = np.zeros((1, MID + OUT + B), dtype=ml_dtypes.bfloat16)
            auxk[0, :MID] = np.asarray(b1[k], dtype=np.float32)
            auxk[0, MID : MID + OUT] = np.asarray(b2[k], dtype=np.float32)
            auxk[0, MID + OUT :] = 1.0
            m["aux"] = auxk
        maps.append(m)
    return maps


def kernel(x, W1, b1, W2, b2, cat_ids) -> np.ndarray:
    with_bias = bool(
        np.any(np.asarray(b1, dtype=np.float32))
        or np.any(np.asarray(b2, dtype=np.float32))
    )
    nc = _get_nc(with_bias)
    in_maps = _make_in_maps(x, W1, b1, W2, b2, with_bias)
    res = run_bass_kernel_spmd(nc, in_maps, list(range(NUM_CAT))).results
    per_cat = np.stack(
        [np.asarray(res[k]["out"], dtype=np.float32) for k in range(NUM_CAT)]
    )  # [8, OUT, B]
    cat = np.asarray(cat_ids).astype(np.int64).reshape(B)
    sel = per_cat[cat, :, np.arange(B)]  # [B, OUT] (gather undoes the transpose)
    return np.ascontiguousarray(sel.reshape(B, 16, 32).astype(np.float32))
